# revision 7
# baseline (speedup 1.0000x reference)
"""Trainium2 Bass kernel: multi-head attention (b=4, s=2048, d_model=1024, h=16).

Sharding over 8 NeuronCores: 2-D (batch x head-half).
  core c -> batch c//2, head group c%2 (8 of 16 heads, qkv dims 512*g..512*g+512).
Per core: QKV column-parallel, per-head attention (scores computed transposed,
softmax sums via a ones-column appended to V in the PV matmul, max-subtraction
skipped -- scores are O(5) so exp is safe), then a pairwise AllGather of the
normalized per-head outputs and a column-parallel output projection.

All matmul operands are bf16 (fp32 PSUM accumulation). The host pre-transposes
x to x^T and lays it out chunk-major ([DKT*NB, 128, 512] blocks) so every x
DMA has a fully contiguous source.

v2 changes vs v1:
  - Ramp: DMA order (wq, wk, x-nb0 first) and hp0's q/k projection groups
    interleaved into the first attention unit so the exp stream starts ~10us
    in instead of ~55us.
  - Progressive output projection: Wo is loaded up front and each (head-pair,
    seq-half) contributes its 2 matmuls per token-tile as soon as its
    AllGather lands, accumulating in an SBUF fp32 buffer (DVE adds). Only the
    last head-pair's second half remains after attention ends, shrinking the
    tail from ~66us to ~15us and filling PE idle in the scalar-bound phase.

Host assembly: out[b] = concat(core 2b cols 0:512, core 2b+1 cols 512:1024).

Self-contained: hardcodes all shapes; builds/compiles once per process.
"""

from contextlib import ExitStack

import ml_dtypes
import numpy as np

import concourse.bass as bass
import concourse.mybir as mybir
import concourse.tile as tile
from concourse import bacc
from concourse.bass_utils import run_bass_kernel_spmd

FP = mybir.dt.float32
BF = mybir.dt.bfloat16
AFT = mybir.ActivationFunctionType
ts = bass.ts

NCORES = 8
D = 1024           # d_model
HD = 64            # head dim
HPC = 8            # heads per core
DQ = HPC * HD      # per-core qkv width = 512
SCALE = 1.0 / np.sqrt(HD)


def emit_mha(nc, tc, io, S):
    """Emit the per-core MHA program. io: dict of DRAM APs."""
    NHP = HPC // 2       # head pairs = 4
    KT = S // 128        # sk tiles
    SQB = S // 512       # sq blocks of 512
    DKT = D // 128       # d_in tiles = 8
    MQ = DQ // 128       # qkv dout tiles = 4
    TT = S // 128        # token tiles
    NB = S // 512        # token blocks of 512

    xt_in, wq_in, bqk_in, wk_in, wv_in, bv_in, wo_in, bo_in, out_ext = (
        io["xt"], io["wq"], io["bqk"], io["wk"], io["wv"], io["bv"],
        io["wo"], io["bo"], io["out"])

    with ExitStack() as ctx:
        const_pool = ctx.enter_context(tc.tile_pool(name="const", bufs=1))
        dram_pool = ctx.enter_context(tc.tile_pool(name="dram", bufs=1, space="DRAM"))
        # one shared PSUM budget: mm 2 + scores 4 + accA 1 + accB 1 = 8 banks
        mm_psum = ctx.enter_context(
            tc.tile_pool(name="mmps", bufs=2, space="PSUM"))
        sc_psum = ctx.enter_context(
            tc.tile_pool(name="scps", bufs=2, space="PSUM"))
        ac_psum = ctx.enter_context(
            tc.tile_pool(name="acps", bufs=1, space="PSUM"))

        # biases for q/k, host-packed [128, 2*MQ]: col m = bq tile m, MQ+m = bk
        bias_qk = const_pool.tile([128, 2 * MQ], FP, tag="bqk", name="bqk")
        nc.sync.dma_start(bias_qk[:], bqk_in[:, :])

        # bv / bo broadcast tiles [128, DQ]
        bv_bc = const_pool.tile([128, DQ], FP, tag="bvbc", name="bvbc")
        bo_bc = const_pool.tile([128, DQ], FP, tag="bobc", name="bobc")
        with tc.tile_pool(name="btmpp", bufs=1) as btmp_pool:
            btmp = btmp_pool.tile([128, DQ], FP, tag="btmp", name="btmp")
            nc.sync.dma_start(
                btmp[0:1, :], bv_in[:].rearrange("(one f) -> one f", one=1))
            nc.gpsimd.partition_broadcast(bv_bc[:], btmp[0:1, :])
            btmp2 = btmp_pool.tile([128, DQ], FP, tag="btmp2", name="btmp2")
            nc.sync.dma_start(
                btmp2[0:1, :], bo_in[:].rearrange("(one f) -> one f", one=1))
            nc.gpsimd.partition_broadcast(bo_bc[:], btmp2[0:1, :])

        def sum_slot(h, sqb):
            # unit (h, sqb) -> partition 32*(h%4) + 8*(h//4), cols sqb*512.
            # Both heads of a pair share h//4, so the unit's reciprocal can
            # cover rows [32*(2hp%4), +64) and hit both slots.
            return 32 * (h % 4) + 8 * (h // 4), ts(sqb, 512)

        # DRAM bounce + per-(head-pair, seq-half) AllGather in/out (bf16);
        # collective operands must be contiguous
        y_bnc = [[dram_pool.tile([128, S // 2], BF, tag=f"ybounce{hp}_{h2}",
                                 name=f"ybounce{hp}_{h2}")
                  for h2 in range(2)]
                 for hp in range(NHP)]
        y_gath = [[dram_pool.tile([256, S // 2], BF, tag=f"ygather{hp}_{h2}",
                                  name=f"ygather{hp}_{h2}")
                   for h2 in range(2)]
                  for hp in range(NHP)]

        with ExitStack() as phase12:
            qkv_pool = phase12.enter_context(tc.tile_pool(name="qkv", bufs=1))
            yt_pool = phase12.enter_context(tc.tile_pool(name="yt", bufs=1))
            exp_pool = phase12.enter_context(tc.tile_pool(name="exp", bufs=4))
            stage_pool = phase12.enter_context(tc.tile_pool(name="stage", bufs=2))
            oa_pool = phase12.enter_context(tc.tile_pool(name="oa", bufs=1))
            ygs_pool = phase12.enter_context(tc.tile_pool(name="ygs", bufs=2))

            # q^T / k^T, d-major: tile hp holds heads 2hp (parts 0-63), 2hp+1
            qT = [qkv_pool.tile([128, S], BF, tag=f"qT{m}", name=f"qT{m}")
                  for m in range(MQ)]
            kT = [qkv_pool.tile([128, S], BF, tag=f"kT{m}", name=f"kT{m}")
                  for m in range(MQ)]
            # v natural [tok, dout] with a ones column per head
            v_ones = [qkv_pool.tile([128, HPC * (HD + 1)], BF, tag=f"v{t}",
                                    name=f"v{t}")
                      for t in range(TT)]
            # softmax sums / reciprocals, packed 32-partition-aligned
            sums_t = yt_pool.tile([128, SQB * 512], FP, tag="sums",
                                  name="sums")
            recip_t = yt_pool.tile([128, SQB * 512], FP, tag="recip",
                                   name="recip")
            nc.gpsimd.memset(sums_t[:], 1.0)

            # SBUF fp32 accumulators for the progressive output projection
            out_acc = [oa_pool.tile([128, DQ], FP, tag=f"oa{t}",
                                    name=f"oa{t}")
                       for t in range(TT)]
            wot = [qkv_pool.tile([128, DQ], BF, tag=f"wo{k}", name=f"wo{k}")
                   for k in range(2 * MQ)]

            def emit_round(hp, h2):
                # out-proj contribution of head-pair hp for seq-half h2:
                # token tiles h2*8..h2*8+7, k2 blocks {hp (rank0), MQ+hp}.
                ygs0 = ygs_pool.tile([128, S // 2], BF, tag="ygs0",
                                     name="ygs0")
                ygs1 = ygs_pool.tile([128, S // 2], BF, tag="ygs1",
                                     name="ygs1")
                nc.sync.dma_start(ygs0[:, :], y_gath[hp][h2][0:128, :])
                nc.sync.dma_start(ygs1[:, :], y_gath[hp][h2][128:256, :])
                for tj in range(TT // 2):
                    ti = h2 * (TT // 2) + tj
                    po = mm_psum.tile([128, 512], FP, tag="mm", name="mm")
                    nc.tensor.matmul(
                        po[:], lhsT=ygs0[:, ts(tj, 128)], rhs=wot[hp][:],
                        start=True, stop=False, skip_group_check=True)
                    nc.tensor.matmul(
                        po[:], lhsT=ygs1[:, ts(tj, 128)], rhs=wot[MQ + hp][:],
                        start=False, stop=True, skip_group_check=True)
                    if hp == 0:
                        nc.vector.tensor_add(out_acc[ti][:], po[:], bo_bc[:])
                    else:
                        nc.vector.tensor_add(out_acc[ti][:], po[:],
                                             out_acc[ti][:])
                    if hp == NHP - 1:
                        nc.sync.dma_start(out_ext[ts(ti, 128), :],
                                          out_acc[ti][:])

            with ExitStack() as phase01:
                # ---- load x^T (chunk-major on host) and weights ----
                xtw_pool = phase01.enter_context(tc.tile_pool(name="xtw", bufs=1))
                xTall = xtw_pool.tile([128, DKT * S], BF, tag="xTall",
                                      name="xTall")
                xT3 = xTall[:].rearrange("p (d s) -> p d s", s=S)

                def xTs(k, sl):
                    return xT3[:, k, sl]

                # full row-blocks [128, DQ] -> large contiguous descriptors
                wv_t = [xtw_pool.tile([128, DQ], BF, tag=f"wv{k}",
                                      name=f"wv{k}")
                        for k in range(DKT)]
                wq_t = [xtw_pool.tile([128, DQ], BF, tag=f"wq{k}",
                                      name=f"wq{k}")
                        for k in range(DKT)]
                wk_t = [xtw_pool.tile([128, DQ], BF, tag=f"wk{k}",
                                      name=f"wk{k}")
                        for k in range(DKT)]

                def load_x_chunk(k, nb):
                    # host chunk-major: chunk (k, nb) is contiguous in DRAM
                    r0 = (k * NB + nb) * 128
                    nc.sync.dma_start(xT3[:, k, ts(nb, 512)],
                                      xt_in[r0:r0 + 128, :])

                # load order follows first consumption: wq + wk + x chunk 0
                # feed the first q/k projection groups; wv for the JIT v; wo
                # for the progressive out-projection; x nb1-3 trickle in.
                for k in range(DKT):
                    nc.sync.dma_start(wq_t[k][:], wq_in[ts(k, 128), :])
                for k in range(DKT):
                    nc.sync.dma_start(wk_t[k][:], wk_in[ts(k, 128), :])
                for k in range(DKT):
                    load_x_chunk(k, 0)
                for k in range(DKT):
                    nc.sync.dma_start(wv_t[k][:], wv_in[ts(k, 128), :])
                for k in range(DKT):
                    load_x_chunk(k, 1)
                for k2 in range(2 * MQ):
                    nc.sync.dma_start(wot[k2][:], wo_in[ts(k2, 128), :])
                for nb in range(2, NB):
                    for k in range(DKT):
                        load_x_chunk(k, nb)

                def emit_qk_group(m, g):
                    # one q/k projection psum group for head-pair m;
                    # g//NB selects q vs k, g%NB the token block. Evac on
                    # DVE (keeps the scalar engine free for the exp stream).
                    w_t, bcol, dstT = ((wq_t, 0, qT), (wk_t, 1, kT))[g // NB]
                    nb = g % NB
                    ps = mm_psum.tile([128, 512], FP, tag="mm", name="mm")
                    for k in range(DKT):
                        nc.tensor.matmul(
                            ps[:], lhsT=w_t[k][:, ts(m, 128)],
                            rhs=xTs(k, ts(nb, 512)),
                            start=(k == 0), stop=(k == DKT - 1))
                    col = bcol * MQ + m
                    nc.vector.tensor_scalar_add(
                        dstT[m][:, ts(nb, 512)], ps[:],
                        bias_qk[:, col:col + 1])

                def emit_v(t0, t1):
                    for ti in range(t0, t1):
                        ps = mm_psum.tile([128, DQ], FP, tag="mm", name="mm")
                        for k in range(DKT):
                            nc.tensor.matmul(
                                ps[:], lhsT=xTs(k, ts(ti, 128)), rhs=wv_t[k][:],
                                start=(k == 0), stop=(k == DKT - 1))
                        vt3 = v_ones[ti][:].rearrange("p (h u) -> p h u",
                                                      u=HD + 1)
                        nc.vector.tensor_add(
                            vt3[:, :, 0:HD],
                            ps[:].rearrange("p (h u) -> p h u", u=HD),
                            bv_bc[:].rearrange("p (h u) -> p h u", u=HD))
                        nc.gpsimd.memset(vt3[:, :, HD:HD + 1], 1.0)

                # ---- per head-pair: attention with hp0's q/k projections
                # interleaved into the first unit so exp starts asap ----
                for hp in range(NHP):
                    if hp == 0:
                        emit_qk_group(0, 0)       # q nb0
                        emit_qk_group(0, NB)      # k nb0

                    # y^T for this head-pair (d-major, normalized in place
                    # per unit); ring of 2 so hp+1 overlaps hp's shipping
                    yTh = yt_pool.tile([128, S], BF, tag="yT", bufs=2,
                                       name="yT")
                    hA, hB = 2 * hp, 2 * hp + 1
                    for sqb in range(SQB):
                        if hp == 0 and sqb > 0:
                            emit_qk_group(0, sqb)  # q nb<sqb>
                        sq = ts(sqb, 512)
                        accA = ac_psum.tile([HD + 1, 512], FP, tag="accA",
                                            name="accA")
                        accB = ac_psum.tile([HD + 1, 512], FP, tag="accB",
                                            name="accB")
                        for k in range(KT):
                            if hp == 0 and sqb == 0 and k % 4 == 0 and k > 0:
                                emit_qk_group(0, NB + k // 4)  # k nb<k//4>
                            sk = ts(k, 128)
                            ps = sc_psum.tile([128, 1024], FP, tag="sc", name="sc")
                            # scores^T [sk, sq] for both heads (row-tiled pair)
                            nc.tensor.matmul(
                                ps[:, 0:512], lhsT=kT[hp][0:64, sk],
                                rhs=qT[hp][0:64, sq], start=True, stop=True)
                            nc.tensor.matmul(
                                ps[:, 512:1024], lhsT=kT[hp][64:128, sk],
                                rhs=qT[hp][64:128, sq], start=True, stop=True)
                            et = exp_pool.tile([128, 1024], BF, tag="exp",
                                               name="exp")
                            nc.scalar.activation(et[:], ps[:], AFT.Exp,
                                                 scale=SCALE)
                            if hp == 0 and sqb == 0:
                                # produce v[k] just in time for its attnv
                                emit_v(k, k + 1)
                            # y^T accumulation: lhsT = [v_h | 1]
                            nc.tensor.matmul(
                                accA[:], lhsT=v_ones[k][:, hA * 65:hA * 65 + 65],
                                rhs=et[:, 0:512],
                                start=(k == 0), stop=(k == KT - 1),
                                skip_group_check=True)
                            nc.tensor.matmul(
                                accB[:], lhsT=v_ones[k][:, hB * 65:hB * 65 + 65],
                                rhs=et[:, 512:1024],
                                start=(k == 0), stop=(k == KT - 1),
                                skip_group_check=True)
                        # extract y (rows 0-63) and sums (row 64)
                        nc.vector.tensor_copy(yTh[0:64, sq], accA[0:64, :])
                        st = stage_pool.tile([128, 512], BF, tag="bst", name="bst")
                        nc.vector.tensor_copy(st[0:64, :], accB[0:64, :])
                        nc.sync.dma_start(yTh[64:128, sq], st[0:64, :])
                        for acc, h in ((accA, hA), (accB, hB)):
                            sp, sc = sum_slot(h, sqb)
                            sA = stage_pool.tile([128, 512], FP, tag="sst",
                                                 name="sst")
                            nc.vector.tensor_copy(sA[64:65, :], acc[64:65, :])
                            nc.sync.dma_start(sums_t[sp:sp + 1, sc],
                                              sA[64:65, :])
                        # reciprocal for this unit (both heads share a
                        # 64-partition band and column slot)
                        band = 32 * (hA % 4)
                        _, sc = sum_slot(hA, sqb)
                        nc.vector.reciprocal(
                            recip_t[band:band + 64, sc],
                            sums_t[band:band + 64, sc])
                        # normalize y^T for this unit in place
                        for h2, h in ((0, hA), (1, hB)):
                            rows = slice(64 * h2, 64 * h2 + 64)
                            sp, _ = sum_slot(h, sqb)
                            # HW partition_broadcast reads partition 0 of the
                            # tensor regardless of the AP base -> stage the
                            # recip row to partition 0 (cast to bf16) first.
                            rtmp = stage_pool.tile([128, 512], FP, tag="rtmp",
                                                   name="rtmp")
                            nc.sync.dma_start(rtmp[0:1, :],
                                              recip_t[sp:sp + 1, sc])
                            rtb = stage_pool.tile([128, 512], BF, tag="rtb",
                                                  name="rtb")
                            nc.vector.tensor_copy(rtb[0:1, :], rtmp[0:1, :])
                            rb = stage_pool.tile([128, 512], BF, tag="rb",
                                                 name="rb")
                            nc.gpsimd.partition_broadcast(rb[:], rtb[0:1, :])
                            nc.vector.tensor_mul(
                                yTh[rows, sq], yTh[rows, sq],
                                rb[rows, :])
                        # drip the next head-pair's projections between units
                        if hp + 1 < NHP:
                            gpu = 2 * NB // SQB
                            for g in range(gpu * sqb, gpu * (sqb + 1)):
                                emit_qk_group(hp + 1, g)
                        # ship and AllGather each completed seq half of this
                        # hp's y^T, then emit its out-projection round
                        # (overlaps remaining attention)
                        covered = (sqb + 1) * 512
                        for h2 in range(2):
                            end = (h2 + 1) * (S // 2)
                            if covered >= end > covered - 512:
                                half = ts(h2, S // 2)
                                nc.sync.dma_start(y_bnc[hp][h2][:, :],
                                                  yTh[:, half])
                                nc.gpsimd.collective_compute(
                                    "AllGather", mybir.AluOpType.bypass,
                                    replica_groups=[[0, 1], [2, 3],
                                                    [4, 5], [6, 7]],
                                    ins=[y_bnc[hp][h2][:, :]],
                                    outs=[y_gath[hp][h2][:, :]])
                                emit_round(hp, h2)


def build_program(S=2048):
    nc = bacc.Bacc(
        "TRN2",
        target_bir_lowering=False,
        debug=False,
        enable_asserts=True,
        num_devices=NCORES,
    )
    NB = S // 512
    DKT = D // 128
    io = {
        "xt": nc.declare_dram_parameter("xt", [DKT * NB * 128, 512], BF,
                                        isOutput=False),
        "wq": nc.declare_dram_parameter("wq", [D, DQ], BF, isOutput=False),
        "bqk": nc.declare_dram_parameter("bqk", [128, 8], FP, isOutput=False),
        "wk": nc.declare_dram_parameter("wk", [D, DQ], BF, isOutput=False),
        "wv": nc.declare_dram_parameter("wv", [D, DQ], BF, isOutput=False),
        "bv": nc.declare_dram_parameter("bv", [DQ], FP, isOutput=False),
        "wo": nc.declare_dram_parameter("wo", [D, DQ], BF, isOutput=False),
        "bo": nc.declare_dram_parameter("bo", [DQ], FP, isOutput=False),
        "out": nc.declare_dram_parameter("out", [S, DQ], FP, isOutput=True),
    }
    io = {k: (v[:] if not isinstance(v, bass.AP) else v) for k, v in io.items()}
    with tile.TileContext(nc) as tc:
        emit_mha(nc, tc, io, S)
    nc.finalize()
    return nc


def shard_inputs(x, Wq, bq, Wk, bk, Wv, bv, Wo, bo):
    """Full inputs -> per-core in_maps. Matmul operands cast to bf16; x is
    transposed and laid out chunk-major on the host so every device DMA has
    a contiguous source."""
    BFNP = ml_dtypes.bfloat16
    S = x.shape[1]
    NB = S // 512
    DKT = D // 128
    f32 = lambda a: np.ascontiguousarray(np.asarray(a), dtype=np.float32)
    bf = lambda a: np.ascontiguousarray(np.asarray(a, dtype=np.float32)
                                        .astype(BFNP))
    x = np.asarray(x, dtype=np.float32).astype(BFNP)
    # x[b].T [D, S] -> chunk-major [(DKT*NB)*128, 512]
    xts = [np.ascontiguousarray(
        x[b].T.reshape(DKT, 128, NB, 512).transpose(0, 2, 1, 3)
        .reshape(DKT * NB * 128, 512)) for b in range(4)]
    Wq, Wk, Wv, Wo = bf(Wq), bf(Wk), bf(Wv), bf(Wo)
    bq, bk, bv, bo = f32(bq), f32(bk), f32(bv), f32(bo)
    in_maps = []
    for c in range(NCORES):
        b, g = divmod(c, 2)
        sl = slice(g * DQ, (g + 1) * DQ)
        bqk = np.empty((128, 8), np.float32)
        for m in range(4):
            bqk[:, m] = bq[sl][m * 128:(m + 1) * 128]
            bqk[:, 4 + m] = bk[sl][m * 128:(m + 1) * 128]
        in_maps.append({
            "xt": xts[b],
            "wq": np.ascontiguousarray(Wq[:, sl]), "bqk": bqk,
            "wk": np.ascontiguousarray(Wk[:, sl]),
            "wv": np.ascontiguousarray(Wv[:, sl]), "bv": bv[sl].copy(),
            "wo": np.ascontiguousarray(Wo[:, sl]), "bo": bo[sl].copy(),
        })
    return in_maps


_CACHE = {}


def _get_program(S=2048):
    if S not in _CACHE:
        _CACHE[S] = build_program(S)
    return _CACHE[S]


def kernel(x, Wq, bq, Wk, bk, Wv, bv, Wo, bo):
    nc = _get_program(2048)
    in_maps = shard_inputs(x, Wq, bq, Wk, bk, Wv, bv, Wo, bo)
    res = run_bass_kernel_spmd(nc, in_maps, list(range(NCORES))).results
    S = 2048
    out = np.empty((4, S, D), dtype=np.float32)
    for c in range(NCORES):
        b, g = divmod(c, 2)
        out[b, :, g * DQ:(g + 1) * DQ] = res[c]["out"]
    return out


# revision 10
# speedup vs baseline: 1.0097x; 1.0097x over previous
"""Trainium2 Bass kernel: multi-head attention (b=4, s=2048, d_model=1024, h=16).

Sharding over 8 NeuronCores: 2-D (batch x head-half).
  core c -> batch c//2, head group c%2 (8 of 16 heads, qkv dims 512*g..512*g+512).
Per core: QKV column-parallel, per-head attention (scores computed transposed,
softmax sums via a ones-column appended to V in the PV matmul, max-subtraction
skipped -- scores are O(5) so exp is safe), then a pairwise AllGather of the
normalized per-head outputs and a column-parallel output projection.

All matmul operands are bf16 (fp32 PSUM accumulation). The host pre-transposes
x to x^T and lays it out chunk-major ([DKT*NB, 128, 512] blocks) so every x
DMA has a fully contiguous source.

v2 changes vs v1:
  - Ramp: DMA order (wq, wk, x-nb0 first) and hp0's q/k projection groups
    interleaved into the first attention unit so the exp stream starts ~10us
    in instead of ~55us.
  - Progressive output projection: Wo is loaded up front and each (head-pair,
    seq-half) contributes its 2 matmuls per token-tile as soon as its
    AllGather lands, accumulating in an SBUF fp32 buffer (DVE adds). Only the
    last head-pair's second half remains after attention ends, shrinking the
    tail from ~66us to ~15us and filling PE idle in the scalar-bound phase.

Host assembly: out[b] = concat(core 2b cols 0:512, core 2b+1 cols 512:1024).

Self-contained: hardcodes all shapes; builds/compiles once per process.
"""

from contextlib import ExitStack

import ml_dtypes
import numpy as np

import concourse.bass as bass
import concourse.mybir as mybir
import concourse.tile as tile
from concourse import bacc
from concourse.bass_utils import run_bass_kernel_spmd

FP = mybir.dt.float32
BF = mybir.dt.bfloat16
AFT = mybir.ActivationFunctionType
ts = bass.ts

NCORES = 8
D = 1024           # d_model
HD = 64            # head dim
HPC = 8            # heads per core
DQ = HPC * HD      # per-core qkv width = 512
SCALE = 1.0 / np.sqrt(HD)


def emit_mha(nc, tc, io, S):
    """Emit the per-core MHA program. io: dict of DRAM APs."""
    NHP = HPC // 2       # head pairs = 4
    KT = S // 128        # sk tiles
    SQB = S // 512       # sq blocks of 512
    DKT = D // 128       # d_in tiles = 8
    MQ = DQ // 128       # qkv dout tiles = 4
    TT = S // 128        # token tiles
    NB = S // 512        # token blocks of 512

    xt_in, wq_in, bqk_in, wk_in, wv_in, bv_in, wo_in, bo_in, out_ext = (
        io["xt"], io["wq"], io["bqk"], io["wk"], io["wv"], io["bv"],
        io["wo"], io["bo"], io["out"])

    with ExitStack() as ctx:
        const_pool = ctx.enter_context(tc.tile_pool(name="const", bufs=1))
        dram_pool = ctx.enter_context(tc.tile_pool(name="dram", bufs=1, space="DRAM"))
        # one shared PSUM budget: mm 2 + scores 4 + accA 1 + accB 1 = 8 banks
        mm_psum = ctx.enter_context(
            tc.tile_pool(name="mmps", bufs=2, space="PSUM"))
        sc_psum = ctx.enter_context(
            tc.tile_pool(name="scps", bufs=2, space="PSUM"))
        ac_psum = ctx.enter_context(
            tc.tile_pool(name="acps", bufs=1, space="PSUM"))

        # biases for q/k, host-packed [128, 2*MQ]: col m = bq tile m, MQ+m = bk
        bias_qk = const_pool.tile([128, 2 * MQ], FP, tag="bqk", name="bqk")
        nc.sync.dma_start(bias_qk[:], bqk_in[:, :])

        # bv / bo broadcast tiles [128, DQ]
        bv_bc = const_pool.tile([128, DQ], FP, tag="bvbc", name="bvbc")
        bo_bc = const_pool.tile([128, DQ], FP, tag="bobc", name="bobc")
        with tc.tile_pool(name="btmpp", bufs=1) as btmp_pool:
            btmp = btmp_pool.tile([128, DQ], FP, tag="btmp", name="btmp")
            nc.sync.dma_start(
                btmp[0:1, :], bv_in[:].rearrange("(one f) -> one f", one=1))
            nc.gpsimd.partition_broadcast(bv_bc[:], btmp[0:1, :])
            btmp2 = btmp_pool.tile([128, DQ], FP, tag="btmp2", name="btmp2")
            nc.sync.dma_start(
                btmp2[0:1, :], bo_in[:].rearrange("(one f) -> one f", one=1))
            nc.gpsimd.partition_broadcast(bo_bc[:], btmp2[0:1, :])

        def sum_slot(h, sqb):
            # unit (h, sqb) -> partition 32*(h%4) + 8*(h//4), cols sqb*512.
            # Both heads of a pair share h//4, so the unit's reciprocal can
            # cover rows [32*(2hp%4), +64) and hit both slots.
            return 32 * (h % 4) + 8 * (h // 4), ts(sqb, 512)

        # DRAM bounce + per-(head-pair, seq-half) AllGather in/out (bf16);
        # collective operands must be contiguous
        y_bnc = [[dram_pool.tile([128, S // 2], BF, tag=f"ybounce{hp}_{h2}",
                                 name=f"ybounce{hp}_{h2}")
                  for h2 in range(2)]
                 for hp in range(NHP)]
        y_gath = [[dram_pool.tile([256, S // 2], BF, tag=f"ygather{hp}_{h2}",
                                  name=f"ygather{hp}_{h2}")
                   for h2 in range(2)]
                  for hp in range(NHP)]

        with ExitStack() as phase12:
            qkv_pool = phase12.enter_context(tc.tile_pool(name="qkv", bufs=1))
            yt_pool = phase12.enter_context(tc.tile_pool(name="yt", bufs=1))
            exp_pool = phase12.enter_context(tc.tile_pool(name="exp", bufs=4))
            stage_pool = phase12.enter_context(tc.tile_pool(name="stage", bufs=2))
            oa_pool = phase12.enter_context(tc.tile_pool(name="oa", bufs=1))
            ygs_pool = phase12.enter_context(tc.tile_pool(name="ygs", bufs=2))

            # q^T / k^T, d-major: tile hp holds heads 2hp (parts 0-63), 2hp+1
            qT = [qkv_pool.tile([128, S], BF, tag=f"qT{m}", name=f"qT{m}")
                  for m in range(MQ)]
            kT = [qkv_pool.tile([128, S], BF, tag=f"kT{m}", name=f"kT{m}")
                  for m in range(MQ)]
            # v natural [tok, dout] with a ones column per head
            v_ones = [qkv_pool.tile([128, HPC * (HD + 1)], BF, tag=f"v{t}",
                                    name=f"v{t}")
                      for t in range(TT)]
            # softmax sums / reciprocals, packed 32-partition-aligned
            sums_t = yt_pool.tile([128, SQB * 512], FP, tag="sums",
                                  name="sums")
            recip_t = yt_pool.tile([128, SQB * 512], FP, tag="recip",
                                   name="recip")
            nc.gpsimd.memset(sums_t[:], 1.0)

            # SBUF fp32 accumulators for the progressive output projection
            out_acc = [oa_pool.tile([128, DQ], FP, tag=f"oa{t}",
                                    name=f"oa{t}")
                       for t in range(TT)]
            wot = [qkv_pool.tile([128, DQ], BF, tag=f"wo{k}", name=f"wo{k}")
                   for k in range(2 * MQ)]

            def emit_round(hp, h2):
                # out-proj contribution of head-pair hp for seq-half h2:
                # token tiles h2*8..h2*8+7, k2 blocks {hp (rank0), MQ+hp}.
                ygs0 = ygs_pool.tile([128, S // 2], BF, tag="ygs0",
                                     name="ygs0")
                ygs1 = ygs_pool.tile([128, S // 2], BF, tag="ygs1",
                                     name="ygs1")
                nc.sync.dma_start(ygs0[:, :], y_gath[hp][h2][0:128, :])
                nc.sync.dma_start(ygs1[:, :], y_gath[hp][h2][128:256, :])
                for tj in range(TT // 2):
                    ti = h2 * (TT // 2) + tj
                    po = mm_psum.tile([128, 512], FP, tag="mm", name="mm")
                    nc.tensor.matmul(
                        po[:], lhsT=ygs0[:, ts(tj, 128)], rhs=wot[hp][:],
                        start=True, stop=False, skip_group_check=True)
                    nc.tensor.matmul(
                        po[:], lhsT=ygs1[:, ts(tj, 128)], rhs=wot[MQ + hp][:],
                        start=False, stop=True, skip_group_check=True)
                    if hp == 0:
                        nc.vector.tensor_add(out_acc[ti][:], po[:], bo_bc[:])
                    else:
                        nc.vector.tensor_add(out_acc[ti][:], po[:],
                                             out_acc[ti][:])
                    if hp == NHP - 1:
                        nc.sync.dma_start(out_ext[ts(ti, 128), :],
                                          out_acc[ti][:])

            with ExitStack() as phase01:
                # ---- load x^T (chunk-major on host) and weights ----
                xtw_pool = phase01.enter_context(tc.tile_pool(name="xtw", bufs=1))
                xTall = xtw_pool.tile([128, DKT * S], BF, tag="xTall",
                                      name="xTall")
                xT3 = xTall[:].rearrange("p (d s) -> p d s", s=S)

                def xTs(k, sl):
                    return xT3[:, k, sl]

                # full row-blocks [128, DQ] -> large contiguous descriptors
                wv_t = [xtw_pool.tile([128, DQ], BF, tag=f"wv{k}",
                                      name=f"wv{k}")
                        for k in range(DKT)]
                wq_t = [xtw_pool.tile([128, DQ], BF, tag=f"wq{k}",
                                      name=f"wq{k}")
                        for k in range(DKT)]
                wk_t = [xtw_pool.tile([128, DQ], BF, tag=f"wk{k}",
                                      name=f"wk{k}")
                        for k in range(DKT)]

                def load_x_chunk(k, nb):
                    # host chunk-major: chunk (k, nb) is contiguous in DRAM
                    r0 = (k * NB + nb) * 128
                    nc.sync.dma_start(xT3[:, k, ts(nb, 512)],
                                      xt_in[r0:r0 + 128, :])

                # load order follows first consumption: wq + wk + x chunk 0
                # feed the first q/k projection groups; wv for the JIT v; wo
                # for the progressive out-projection; x nb1-3 trickle in.
                for k in range(DKT):
                    nc.sync.dma_start(wq_t[k][:], wq_in[ts(k, 128), :])
                for k in range(DKT):
                    nc.sync.dma_start(wk_t[k][:], wk_in[ts(k, 128), :])
                for k in range(DKT):
                    load_x_chunk(k, 0)
                for k in range(DKT):
                    nc.sync.dma_start(wv_t[k][:], wv_in[ts(k, 128), :])
                for k in range(DKT):
                    load_x_chunk(k, 1)
                for k2 in range(2 * MQ):
                    nc.sync.dma_start(wot[k2][:], wo_in[ts(k2, 128), :])
                for nb in range(2, NB):
                    for k in range(DKT):
                        load_x_chunk(k, nb)

                def emit_qk_group(m, g):
                    # one q/k projection psum group for head-pair m;
                    # g//NB selects q vs k, g%NB the token block. Evac on
                    # DVE (keeps the scalar engine free for the exp stream).
                    w_t, bcol, dstT = ((wq_t, 0, qT), (wk_t, 1, kT))[g // NB]
                    nb = g % NB
                    ps = mm_psum.tile([128, 512], FP, tag="mm", name="mm")
                    for k in range(DKT):
                        nc.tensor.matmul(
                            ps[:], lhsT=w_t[k][:, ts(m, 128)],
                            rhs=xTs(k, ts(nb, 512)),
                            start=(k == 0), stop=(k == DKT - 1))
                    col = bcol * MQ + m
                    nc.vector.tensor_scalar_add(
                        dstT[m][:, ts(nb, 512)], ps[:],
                        bias_qk[:, col:col + 1])

                def emit_v(t0, t1):
                    for ti in range(t0, t1):
                        ps = mm_psum.tile([128, DQ], FP, tag="mm", name="mm")
                        for k in range(DKT):
                            nc.tensor.matmul(
                                ps[:], lhsT=xTs(k, ts(ti, 128)), rhs=wv_t[k][:],
                                start=(k == 0), stop=(k == DKT - 1))
                        vt3 = v_ones[ti][:].rearrange("p (h u) -> p h u",
                                                      u=HD + 1)
                        nc.vector.tensor_add(
                            vt3[:, :, 0:HD],
                            ps[:].rearrange("p (h u) -> p h u", u=HD),
                            bv_bc[:].rearrange("p (h u) -> p h u", u=HD))
                        nc.gpsimd.memset(vt3[:, :, HD:HD + 1], 1.0)

                # ---- per head-pair: attention with hp0's q/k projections
                # interleaved into the first unit so exp starts asap ----
                # out-proj rounds are deferred by >=1 unit so their AllGather
                # has completed before their PSUM tile reaches the front of
                # the shared "mm" ring (else the ring stalls the projection
                # pipeline behind the collective).
                pending_rounds = []

                def flush_rounds(unit_now):
                    while pending_rounds and pending_rounds[0][2] < unit_now:
                        rhp, rh2, _ = pending_rounds.pop(0)
                        emit_round(rhp, rh2)

                for hp in range(NHP):
                    if hp == 0:
                        emit_qk_group(0, 0)       # q nb0
                        emit_qk_group(0, NB)      # k nb0

                    # y^T for this head-pair (d-major, normalized in place
                    # per unit); ring of 2 so hp+1 overlaps hp's shipping
                    yTh = yt_pool.tile([128, S], BF, tag="yT", bufs=2,
                                       name="yT")
                    hA, hB = 2 * hp, 2 * hp + 1
                    for sqb in range(SQB):
                        flush_rounds(hp * SQB + sqb)
                        if hp == 0 and sqb > 0:
                            emit_qk_group(0, sqb)  # q nb<sqb>
                        sq = ts(sqb, 512)
                        accA = ac_psum.tile([HD + 1, 512], FP, tag="accA",
                                            name="accA")
                        accB = ac_psum.tile([HD + 1, 512], FP, tag="accB",
                                            name="accB")
                        for k in range(KT):
                            if hp == 0 and sqb == 0 and k % 4 == 0 and k > 0:
                                emit_qk_group(0, NB + k // 4)  # k nb<k//4>
                            sk = ts(k, 128)
                            ps = sc_psum.tile([128, 1024], FP, tag="sc", name="sc")
                            # scores^T [sk, sq] for both heads (row-tiled pair)
                            nc.tensor.matmul(
                                ps[:, 0:512], lhsT=kT[hp][0:64, sk],
                                rhs=qT[hp][0:64, sq], start=True, stop=True)
                            nc.tensor.matmul(
                                ps[:, 512:1024], lhsT=kT[hp][64:128, sk],
                                rhs=qT[hp][64:128, sq], start=True, stop=True)
                            et = exp_pool.tile([128, 1024], BF, tag="exp",
                                               name="exp")
                            nc.scalar.activation(et[:], ps[:], AFT.Exp,
                                                 scale=SCALE)
                            if hp == 0 and sqb == 0:
                                # produce v[k] just in time for its attnv
                                emit_v(k, k + 1)
                            # y^T accumulation: lhsT = [v_h | 1]
                            nc.tensor.matmul(
                                accA[:], lhsT=v_ones[k][:, hA * 65:hA * 65 + 65],
                                rhs=et[:, 0:512],
                                start=(k == 0), stop=(k == KT - 1),
                                skip_group_check=True)
                            nc.tensor.matmul(
                                accB[:], lhsT=v_ones[k][:, hB * 65:hB * 65 + 65],
                                rhs=et[:, 512:1024],
                                start=(k == 0), stop=(k == KT - 1),
                                skip_group_check=True)
                        # extract y (rows 0-63) and sums (row 64)
                        nc.vector.tensor_copy(yTh[0:64, sq], accA[0:64, :])
                        st = stage_pool.tile([128, 512], BF, tag="bst", name="bst")
                        nc.vector.tensor_copy(st[0:64, :], accB[0:64, :])
                        nc.sync.dma_start(yTh[64:128, sq], st[0:64, :])
                        for acc, h in ((accA, hA), (accB, hB)):
                            sp, sc = sum_slot(h, sqb)
                            sA = stage_pool.tile([128, 512], FP, tag="sst",
                                                 name="sst")
                            nc.vector.tensor_copy(sA[64:65, :], acc[64:65, :])
                            nc.sync.dma_start(sums_t[sp:sp + 1, sc],
                                              sA[64:65, :])
                        # reciprocal for this unit (both heads share a
                        # 64-partition band and column slot)
                        band = 32 * (hA % 4)
                        _, sc = sum_slot(hA, sqb)
                        nc.vector.reciprocal(
                            recip_t[band:band + 64, sc],
                            sums_t[band:band + 64, sc])
                        # normalize y^T for this unit in place
                        for h2, h in ((0, hA), (1, hB)):
                            rows = slice(64 * h2, 64 * h2 + 64)
                            sp, _ = sum_slot(h, sqb)
                            # HW partition_broadcast reads partition 0 of the
                            # tensor regardless of the AP base -> stage the
                            # recip row to partition 0 (cast to bf16) first.
                            rtmp = stage_pool.tile([128, 512], FP, tag="rtmp",
                                                   name="rtmp")
                            nc.sync.dma_start(rtmp[0:1, :],
                                              recip_t[sp:sp + 1, sc])
                            rtb = stage_pool.tile([128, 512], BF, tag="rtb",
                                                  name="rtb")
                            nc.vector.tensor_copy(rtb[0:1, :], rtmp[0:1, :])
                            rb = stage_pool.tile([128, 512], BF, tag="rb",
                                                 name="rb")
                            nc.gpsimd.partition_broadcast(rb[:], rtb[0:1, :])
                            nc.vector.tensor_mul(
                                yTh[rows, sq], yTh[rows, sq],
                                rb[rows, :])
                        # drip the next head-pair's projections between units
                        if hp + 1 < NHP:
                            gpu = 2 * NB // SQB
                            for g in range(gpu * sqb, gpu * (sqb + 1)):
                                emit_qk_group(hp + 1, g)
                        # ship and AllGather each completed seq half of this
                        # hp's y^T, then emit its out-projection round
                        # (overlaps remaining attention)
                        covered = (sqb + 1) * 512
                        for h2 in range(2):
                            end = (h2 + 1) * (S // 2)
                            if covered >= end > covered - 512:
                                half = ts(h2, S // 2)
                                nc.sync.dma_start(y_bnc[hp][h2][:, :],
                                                  yTh[:, half])
                                nc.gpsimd.collective_compute(
                                    "AllGather", mybir.AluOpType.bypass,
                                    replica_groups=[[0, 1], [2, 3],
                                                    [4, 5], [6, 7]],
                                    ins=[y_bnc[hp][h2][:, :]],
                                    outs=[y_gath[hp][h2][:, :]])
                                pending_rounds.append(
                                    (hp, h2, hp * SQB + sqb))
                # remaining rounds (last head-pair's halves)
                for rhp, rh2, _ in pending_rounds:
                    emit_round(rhp, rh2)


def build_program(S=2048):
    nc = bacc.Bacc(
        "TRN2",
        target_bir_lowering=False,
        debug=False,
        enable_asserts=True,
        num_devices=NCORES,
    )
    NB = S // 512
    DKT = D // 128
    io = {
        "xt": nc.declare_dram_parameter("xt", [DKT * NB * 128, 512], BF,
                                        isOutput=False),
        "wq": nc.declare_dram_parameter("wq", [D, DQ], BF, isOutput=False),
        "bqk": nc.declare_dram_parameter("bqk", [128, 8], FP, isOutput=False),
        "wk": nc.declare_dram_parameter("wk", [D, DQ], BF, isOutput=False),
        "wv": nc.declare_dram_parameter("wv", [D, DQ], BF, isOutput=False),
        "bv": nc.declare_dram_parameter("bv", [DQ], FP, isOutput=False),
        "wo": nc.declare_dram_parameter("wo", [D, DQ], BF, isOutput=False),
        "bo": nc.declare_dram_parameter("bo", [DQ], FP, isOutput=False),
        "out": nc.declare_dram_parameter("out", [S, DQ], FP, isOutput=True),
    }
    io = {k: (v[:] if not isinstance(v, bass.AP) else v) for k, v in io.items()}
    with tile.TileContext(nc) as tc:
        emit_mha(nc, tc, io, S)
    nc.finalize()
    return nc


def shard_inputs(x, Wq, bq, Wk, bk, Wv, bv, Wo, bo):
    """Full inputs -> per-core in_maps. Matmul operands cast to bf16; x is
    transposed and laid out chunk-major on the host so every device DMA has
    a contiguous source."""
    BFNP = ml_dtypes.bfloat16
    S = x.shape[1]
    NB = S // 512
    DKT = D // 128
    f32 = lambda a: np.ascontiguousarray(np.asarray(a), dtype=np.float32)
    bf = lambda a: np.ascontiguousarray(np.asarray(a, dtype=np.float32)
                                        .astype(BFNP))
    x = np.asarray(x, dtype=np.float32).astype(BFNP)
    # x[b].T [D, S] -> chunk-major [(DKT*NB)*128, 512]
    xts = [np.ascontiguousarray(
        x[b].T.reshape(DKT, 128, NB, 512).transpose(0, 2, 1, 3)
        .reshape(DKT * NB * 128, 512)) for b in range(4)]
    Wq, Wk, Wv, Wo = bf(Wq), bf(Wk), bf(Wv), bf(Wo)
    bq, bk, bv, bo = f32(bq), f32(bk), f32(bv), f32(bo)
    in_maps = []
    for c in range(NCORES):
        b, g = divmod(c, 2)
        sl = slice(g * DQ, (g + 1) * DQ)
        bqk = np.empty((128, 8), np.float32)
        for m in range(4):
            bqk[:, m] = bq[sl][m * 128:(m + 1) * 128]
            bqk[:, 4 + m] = bk[sl][m * 128:(m + 1) * 128]
        in_maps.append({
            "xt": xts[b],
            "wq": np.ascontiguousarray(Wq[:, sl]), "bqk": bqk,
            "wk": np.ascontiguousarray(Wk[:, sl]),
            "wv": np.ascontiguousarray(Wv[:, sl]), "bv": bv[sl].copy(),
            "wo": np.ascontiguousarray(Wo[:, sl]), "bo": bo[sl].copy(),
        })
    return in_maps


_CACHE = {}


def _get_program(S=2048):
    if S not in _CACHE:
        _CACHE[S] = build_program(S)
    return _CACHE[S]


def kernel(x, Wq, bq, Wk, bk, Wv, bv, Wo, bo):
    nc = _get_program(2048)
    in_maps = shard_inputs(x, Wq, bq, Wk, bk, Wv, bv, Wo, bo)
    res = run_bass_kernel_spmd(nc, in_maps, list(range(NCORES))).results
    S = 2048
    out = np.empty((4, S, D), dtype=np.float32)
    for c in range(NCORES):
        b, g = divmod(c, 2)
        out[b, :, g * DQ:(g + 1) * DQ] = res[c]["out"]
    return out
